# revision 41
# baseline (speedup 1.0000x reference)
"""Trainium2 Bass kernel for nn_Bone_loss (VarLoss bone-length variance loss).

Strategy (pure data-parallel over 8 cores, 1024 samples each):
  - Sample relabeling s = 8p + g (partition p, within-partition group g in
    [0,8)) makes every per-sample input load a contiguous 128-row DMA.
  - Each sample contributes 14 gathered scalars pred[s, jt] = output[s,
    ind[s,jt]] via hardware-DGE indirect DMA: the gpsimd sequencer expands
    one 4-byte descriptor per int32 offset read from an SBUF table
    (~1.27 ns/entry + ~0.4 us/call, serialized on gpsimd), and the SDMA
    rings execute them (~4.1 ns per EXECUTED descriptor aggregate; skipped
    OOB entries ~0.45 ns).  The offset-table walk is HARDWIRED to 128
    entries per column (desc d reads table partition d%128, column d//128).
  - The offsets tile offs[p, g*14+jt] IS the table ([128 part, 112 slot
    columns], no pads, no pre-transpose): 14336 expanded entries.  Ring
    time is executed-count bound, so offsets are marked BIG (skipped)
    unless the joint is NEEDED: visible (target>0.5) AND >=1 bone partner
    visible (neighbor-max over two isomorphic 7-joint chains) -- ~33.9%
    executed on U[0,1] visibility.
  - 4 pipelined calls of [32,32,32,16] slot-columns (consumption limit is
    32 columns/call); expansion of call k+1 overlaps ring execution of
    call k; landing partitions on distinct SBUF AXI ports.  Each chunk is
    sprayed (512B/partition) into transposed predT[slot, p]; pair-wise PE
    transposes at legal partition bases 0/64 restore pred[p, slot] via
    PSUM (A-pair overlaps the gather; only the 48-slot B-pair is on the
    tail).  PSUM->SBUF copies: A on scalar, B on vector (queued after the
    early work so the in-order vector queue never stalls on it).
  - Early work (masks, gt_2d bone terms, 1/num with the active mask folded
    in) is emitted between the table cast and the gather so it fills the
    vector queue during expansion.  The tail is only pred differences
    (7 affine AP runs), d2 assembly, visibility bit-mask, sqrt (gt overlaps
    on vector), broadcast-subtract group means, and a fused
    scalar_tensor_tensor whose accum_out yields per-partition row sums.
  - Per-core total via PE ones-matmul -> host adds 8 partials, *0.5/B.
"""

import numpy as np

import concourse.bass as bass
import concourse.tile as tile
from concourse import bacc, mybir
from concourse.bass_utils import run_bass_kernel_spmd

NCORES = 8
B = 8192
S = B // NCORES          # samples per core
P = 128
G = 8                    # samples per partition
UJ = 14                  # used joints per sample (no pads)
U = G * UJ               # within-partition slot count = 112
_COLS = [32, 32, 32, 16]  # table-column split per gather call (columns = slots)
_PIDX = [32, 64, 36, 68, 40, 72, 44, 76]   # landing partitions (distinct ports, off the SWDGE ring partitions 0-31)

_JL = [0, 1, 2, 3, 4, 5, 6, 8, 11, 12, 13, 14, 15, 16]      # joints used
# contiguous (jtpos0, joint0, len) runs of used joints
_JRUNS = [(0, 0, 7), (7, 8, 1), (8, 11, 6)]
# Bones reordered within groups so endpoint jtpos sequences form affine runs.
# Groups stay [0:4], [4:8], [8:10], [10:12].
_ID1 = [2, 3, 4, 5, 11, 12, 15, 16, 1, 4, 14, 11]
_ID2 = [1, 2, 5, 6, 12, 13, 14, 15, 0, 0, 8, 8]
_WB = [1.0, 1.0085885098415446, 1.0, 1.0085885098415446,
       1.0, 1.1375361376887123, 1.0, 1.1375361376887123,
       1.0, 1.0, 1.0, 1.0]
# affine runs (bone0, len, jtpos0, stride) per endpoint, for target/gt_2d
_RUNS_E1 = [(0, 4, 2, 1), (4, 2, 8, 1), (6, 2, 12, 1), (8, 1, 1, 1),
            (9, 1, 4, 1), (10, 1, 11, 1), (11, 1, 8, 1)]
_RUNS_E2 = [(0, 2, 1, 1), (2, 2, 5, 1), (4, 4, 9, 1), (8, 2, 0, 0),
            (10, 2, 7, 0)]
# joint endpoint-difference pieces (bone0, len, q1, st1, q2, st2):
# dp[b] = pred[q1+st1*k] - pred[q2+st2*k]
_DP_PIECES = [(0, 2, 2, 1, 1, 1), (2, 2, 4, 1, 5, 1), (4, 2, 8, 1, 9, 1),
              (6, 2, 12, 1, 11, 1), (8, 2, 1, 3, 0, 0),
              (10, 1, 11, 0, 7, 0), (11, 1, 8, 0, 7, 0)]
# neighbor-max build in jtpos space: both 7-joint chains (base 0, base 7)
# have edges (a,a+1),(a+1,a+2),(a+2,a+3),(a+4,a+5),(a+5,a+6),(a,a+4).
# (dst_off, src_off, len, op) with op 'c'=copy, 'm'=max-accumulate
_NB_OPS = [(0, 1, 3, 'c'), (3, 2, 1, 'c'), (4, 5, 2, 'c'), (6, 5, 1, 'c'),
           (1, 0, 2, 'm'), (5, 4, 1, 'm'), (0, 4, 1, 'm'), (4, 0, 1, 'm')]
_VAR_WEIGHT = 1.0
_BIG = 8388608.0         # 2^23: skipped-offset marker, > bounds_check

_F32 = mybir.dt.float32
_I32 = mybir.dt.int32


def _ap(base_ap, dims, off=0):
    """Custom AP: keep base partition dim, override free dims; offset in elems."""
    return bass.AP(base_ap.tensor, base_ap.offset + off,
                   [list(base_ap.ap[0])] + [list(d) for d in dims])


def _dap(base_ap, dims, off=0):
    """Custom DRAM AP with ALL dims explicit (first dim included)."""
    return bass.AP(base_ap.tensor, base_ap.offset + off,
                   [list(d) for d in dims])


def _consts():
    p = np.arange(P, dtype=np.float32)
    g = np.arange(G, dtype=np.float32)
    c_pg = ((p[:, None] * 8 + g[None, :]) * 4096 + _BIG).astype(np.float32)
    c_id = np.eye(P, dtype=np.float32)
    c_w = np.broadcast_to(np.asarray(_WB, np.float32), (P, 12)).copy()
    c_one = np.ones((P, 1), np.float32)
    return {"c_pg": c_pg, "c_id": c_id, "c_w": c_w, "c_one": c_one}


def _build_nc():
    nc = bacc.Bacc("TRN2", target_bir_lowering=False, debug=False,
                   enable_asserts=False, num_devices=NCORES,
                   dynamic_dma_scratch_size=36864, num_swdge_queues=4)
    outv = nc.dram_tensor("outv", [S * 4096, 1], _F32, kind="ExternalInput").ap()
    indv = nc.dram_tensor("indv", [S, 34], _I32, kind="ExternalInput").ap()
    tgtv = nc.dram_tensor("tgtv", [S, 17], _F32, kind="ExternalInput").ap()
    gxyv = nc.dram_tensor("gxyv", [S, 34], _F32, kind="ExternalInput").ap()
    mskv = nc.dram_tensor("mskv", [S, 17], _F32, kind="ExternalInput").ap()
    c_pg = nc.dram_tensor("c_pg", [P, G], _F32, kind="ExternalInput").ap()
    c_id = nc.dram_tensor("c_id", [P, P], _F32, kind="ExternalInput").ap()
    c_w = nc.dram_tensor("c_w", [P, 12], _F32, kind="ExternalInput").ap()
    c_one = nc.dram_tensor("c_one", [P, 1], _F32, kind="ExternalInput").ap()
    res = nc.dram_tensor("res", [1, 1], _F32, kind="ExternalOutput").ap()

    AL = mybir.AluOpType
    X = mybir.AxisListType.X
    with tile.TileContext(nc) as tc:
        with tc.tile_pool(name="sbuf", bufs=1) as pool, \
             tc.tile_pool(name="psum", bufs=1, space="PSUM") as psum_pool:
            # ---- loads: all contiguous 128-row DMAs (s = 8p + g) ----
            tgt_t = pool.tile([P, G * 17], _F32)
            nc.sync.dma_start(tgt_t[:], _dap(tgtv[:], [[136, P], [1, 136]]))
            ind_t = pool.tile([P, G * 34], _I32)
            nc.sync.dma_start(ind_t[:], _dap(indv[:], [[272, P], [1, 272]]))
            pg_t = pool.tile([P, G], _F32)
            nc.scalar.dma_start(pg_t[:], c_pg[:])
            id_t = pool.tile([P, P], _F32)
            nc.scalar.dma_start(id_t[:], c_id[:])
            gxy_t = pool.tile([P, G * 34], _F32)
            nc.scalar.dma_start(gxy_t[:], _dap(gxyv[:], [[272, P], [1, 272]]))
            msk_t = pool.tile([P, G * 17], _F32)
            nc.scalar.dma_start(msk_t[:], _dap(mskv[:], [[136, P], [1, 136]]))
            w_t = pool.tile([P, 12], _F32)
            nc.scalar.dma_start(w_t[:], c_w[:])
            one_t = pool.tile([P, 1], _F32)
            nc.scalar.dma_start(one_t[:], c_one[:])
            # prefetch the Sqrt activation table off the critical tail
            warm = pool.tile([1, 1], _F32)
            nc.vector.memset(warm[:], 1.0)
            nc.scalar.sqrt(out=warm[:], in_=warm[:])

            # ---- offsets in sample layout: offs[p, g*14+jt] ----
            # offs = ind + (s*4096 + BIG) - BIG*needed
            offs = pool.tile([P, U], _F32)
            for (jp0, jo0, ln) in _JRUNS:
                nc.vector.tensor_copy(
                    out=_ap(offs[:], [[UJ, G], [1, ln]], off=jp0),
                    in_=_ap(ind_t[:], [[34, G], [2, ln]], off=2 * jo0))
            nc.vector.tensor_tensor(
                out=_ap(offs[:], [[UJ, G], [1, 14]]),
                in0=_ap(offs[:], [[UJ, G], [1, 14]]),
                in1=_ap(pg_t[:], [[1, G], [0, 14]]), op=AL.add)
            # vt[p, u] = target value of that slot's joint
            vt = pool.tile([P, U], _F32)
            for (jp0, jo0, ln) in _JRUNS:
                nc.gpsimd.tensor_copy(
                    out=_ap(vt[:], [[UJ, G], [1, ln]], off=jp0),
                    in_=_ap(tgt_t[:], [[17, G], [1, ln]], off=jo0))
            # nb = neighbor-max of vt over the two 7-chains in each group;
            # copies on gpsimd (fast 3-dim APs), max-accumulates on vector
            nb = pool.tile([P, U], _F32)
            for (d0, s0, ln, op) in _NB_OPS:
                dst = _ap(nb[:], [[UJ, G], [7, 2], [1, ln]], off=d0)
                src = _ap(vt[:], [[UJ, G], [7, 2], [1, ln]], off=s0)
                if op == 'c':
                    nc.gpsimd.tensor_copy(out=dst, in_=src)
                else:
                    nc.vector.tensor_tensor(out=dst, in0=dst, in1=src,
                                            op=AL.max)
            # needed = (min(nb, vt) > 0.5); offs += needed * -BIG
            nm = pool.tile([P, U], _F32)
            nc.vector.tensor_tensor(out=nm[:], in0=nb[:], in1=vt[:],
                                    op=AL.min)
            nc.vector.tensor_scalar(out=nm[:], in0=nm[:], scalar1=0.5,
                                    scalar2=-_BIG, op0=AL.is_gt, op1=AL.mult)
            nc.vector.tensor_tensor(
                out=_ap(offs[:], [[UJ, G], [1, 14]]),
                in0=_ap(offs[:], [[UJ, G], [1, 14]]),
                in1=_ap(nm[:], [[UJ, G], [1, 14]]), op=AL.add)

            # ---- offset table = offs cast to int32 ([128, 112]; the
            # hardwired 128-tall column walk maps column c to slot c) ----
            table = pool.tile([P, P], _I32)
            nc.vector.tensor_copy(out=table[:, 0:U], in_=offs[:])

            # ---- early work (overlaps gather): masks, gt_2d bone terms ----
            tc.tile_set_cur_wait(0.5)
            msum = pool.tile([P, G], _F32)
            nc.vector.tensor_reduce(out=msum[:],
                                    in_=_ap(msk_t[:], [[17, G], [1, 17]]),
                                    axis=X, op=AL.add)
            nc.vector.tensor_scalar(out=msum[:], in0=msum[:], scalar1=0.0,
                                    scalar2=None, op0=AL.is_equal)

            t1b = pool.tile([P, 96], _F32)
            t2b = pool.tile([P, 96], _F32)
            for e, (runs, dst) in enumerate(((_RUNS_E1, t1b), (_RUNS_E2, t2b))):
                for (b0, ln, q0, st) in runs:
                    nc.vector.tensor_copy(
                        out=_ap(dst[:], [[8, ln], [1, 8]], off=b0 * 8),
                        in_=_ap(tgt_t[:], [[st, ln], [17, 8]], off=_JL[q0]))
            v1 = pool.tile([P, 96], _F32)
            v2 = pool.tile([P, 96], _F32)
            vis = pool.tile([P, 96], _F32)
            nc.vector.tensor_scalar(out=v1[:], in0=t1b[:], scalar1=0.5,
                                    scalar2=None, op0=AL.is_gt)
            nc.vector.tensor_scalar(out=v2[:], in0=t2b[:], scalar1=0.5,
                                    scalar2=None, op0=AL.is_gt)
            nc.vector.tensor_tensor(out=vis[:], in0=v1[:], in1=v2[:], op=AL.mult)
            # visibility as an all-ones/zeros int mask for d2 sanitization
            vmi = pool.tile([P, 96], _I32)
            nc.vector.tensor_copy(out=vmi[:], in_=vis[:])
            nc.vector.tensor_scalar(out=vmi[:], in0=vmi[:], scalar1=-1,
                                    scalar2=None, op0=AL.mult)

            gxyb = pool.tile([P, 384], _F32)   # [e*192 + b*16 + xy*8 + g]
            for e, runs in enumerate((_RUNS_E1, _RUNS_E2)):
                for (b0, ln, q0, st) in runs:
                    nc.vector.tensor_copy(
                        out=_ap(gxyb[:], [[16, ln], [8, 2], [1, 8]],
                                off=e * 192 + b0 * 16),
                        in_=_ap(gxy_t[:], [[2 * st, ln], [1, 2], [34, 8]],
                                off=2 * _JL[q0]))
            dx = pool.tile([P, 96], _F32)
            dy = pool.tile([P, 96], _F32)
            xy2 = pool.tile([P, 96], _F32)
            nc.vector.tensor_tensor(
                out=dx[:].rearrange("p (a b) -> p a b", a=12),
                in0=_ap(gxyb[:], [[16, 12], [1, 8]], off=0),
                in1=_ap(gxyb[:], [[16, 12], [1, 8]], off=192), op=AL.subtract)
            nc.vector.tensor_tensor(
                out=dy[:].rearrange("p (a b) -> p a b", a=12),
                in0=_ap(gxyb[:], [[16, 12], [1, 8]], off=8),
                in1=_ap(gxyb[:], [[16, 12], [1, 8]], off=200), op=AL.subtract)
            nc.vector.tensor_tensor(out=dx[:], in0=dx[:], in1=dx[:], op=AL.mult)
            nc.vector.tensor_tensor(out=dy[:], in0=dy[:], in1=dy[:], op=AL.mult)
            nc.vector.tensor_tensor(out=xy2[:], in0=dx[:], in1=dy[:], op=AL.add)
            # fold w into xy2 (ell = sqrt((w*dp)^2 + w^2*xy2))
            w2 = pool.tile([P, 96], _F32)
            nc.vector.tensor_tensor(
                out=w2[:].rearrange("p (a b) -> p a b", a=12),
                in0=_ap(w_t[:], [[1, 12], [0, 8]]),
                in1=_ap(w_t[:], [[1, 12], [0, 8]]), op=AL.mult)
            nc.vector.tensor_tensor(out=xy2[:], in0=xy2[:], in1=w2[:],
                                    op=AL.mult)
            # group visible-bone counts and 1/max(num,1): vis-only, pred-free
            num = pool.tile([P, 32], _F32)
            nc.vector.tensor_reduce(
                out=_ap(num[:], [[8, 2], [1, 8]]),
                in_=_ap(vis[:], [[32, 2], [1, 8], [8, 4]]),
                axis=X, op=AL.add)
            nc.vector.tensor_reduce(
                out=_ap(num[:], [[8, 2], [1, 8]], off=16),
                in_=_ap(vis[:], [[16, 2], [1, 8], [8, 2]], off=64),
                axis=X, op=AL.add)
            nc.vector.tensor_scalar(out=num[:], in0=num[:], scalar1=1.0,
                                    scalar2=None, op0=AL.max)
            rn = pool.tile([P, 32], _F32)
            nc.vector.reciprocal(out=rn[:], in_=num[:])
            nb_b = pool.tile([P, 96], _F32)
            nc.vector.tensor_copy(
                out=_ap(nb_b[:], [[32, 2], [8, 4], [1, 8]]),
                in_=_ap(rn[:], [[8, 2], [0, 4], [1, 8]]))
            nc.vector.tensor_copy(
                out=_ap(nb_b[:], [[16, 2], [8, 2], [1, 8]], off=64),
                in_=_ap(rn[:], [[8, 2], [0, 2], [1, 8]], off=16))
            # fold the per-sample active mask in now (row-sum absorbs it)
            nc.vector.tensor_tensor(
                out=nb_b[:], in0=nb_b[:],
                in1=_ap(msum[:], [[0, 12], [1, 8]]), op=AL.mult)

            # ---- indirect gather: 4-byte descriptors, pipelined calls ----
            lins = []
            c0 = 0
            for k, cw in enumerate(_COLS):
                chunk = cw * P
                pk = _PIDX[k % len(_PIDX)]
                lin_k = pool.tile([P, chunk], _F32, name=f"lin{k}")
                lins.append((lin_k, pk, c0, cw))
                gi = nc.gpsimd.indirect_dma_start(
                    out=_ap(lin_k[pk:pk + 1, :], [[1, chunk], [1, 1]]),
                    out_offset=None,
                    in_=outv[:],
                    in_offset=bass.IndirectOffsetOnAxis(
                        ap=table[:, c0:c0 + cw], axis=0),
                    bounds_check=S * 4096 - 1,
                    oob_is_err=False,
                )
                gi.ins.single_packet = True
                gi.ins.queue = f"qPoolDynamic{(k % 4) or ''}"
                c0 += cw
            # ---- spray linear landings -> predT[slot, p]; PE-transpose
            # back to PSUM pred[p, slot] in two group-aligned 56-slot pairs
            # (slots 0-55 = groups 0-3, slots 56-111 = groups 4-7) at legal
            # PE partition bases 0 and 64.  Chunk 1's spray is split so the
            # g>=4 slots (56-63) land in the B region. ----
            predT = pool.tile([P, P], _F32)
            ps_a = psum_pool.tile([P, 56], _F32, space="PSUM")
            ps_b = psum_pool.tile([P, 56], _F32, space="PSUM")
            predt = pool.tile([P, U], _F32)
            _SPR = [(0, 0, 32, 0), (1, 0, 24, 32), (1, 24 * P, 8, 64),
                    (2, 0, 32, 72), (3, 0, 16, 104)]
            for (k, so, rows, pb) in _SPR:
                (lin_k, pk, _c0, _cw) = lins[k]
                nc.sync.dma_start(
                    predT[pb:pb + rows, :],
                    _ap(lin_k[pk:pk + 1, :], [[P, rows], [1, P]], off=so))
                if (k, pb) == (1, 64):
                    nc.tensor.transpose(out=ps_a[:], in_=predT[0:56, :],
                                        identity=id_t[0:56, 0:56])
                    nc.scalar.copy(out=predt[:, 0:56], in_=ps_a[:])
                elif k == 3:
                    nc.tensor.transpose(out=ps_b[:], in_=predT[64:120, :],
                                        identity=id_t[64:120, 64:120])
                    nc.vector.tensor_copy(out=predt[:, 56:112], in_=ps_b[:])

            # ---- late bone math (needs pred) ----
            tc.tile_set_cur_wait(1.0)
            dp = pool.tile([P, 96], _F32)
            for (b0, ln, q1, st1, q2, st2) in _DP_PIECES:
                nc.vector.tensor_tensor(
                    out=_ap(dp[:], [[8, ln], [1, 8]], off=b0 * 8),
                    in0=_ap(predt[:], [[st1, ln], [UJ, 8]], off=q1),
                    in1=_ap(predt[:], [[st2, ln], [UJ, 8]], off=q2),
                    op=AL.subtract)
            nc.vector.tensor_tensor(
                out=dp[:].rearrange("p (a b) -> p a b", a=12),
                in0=dp[:].rearrange("p (a b) -> p a b", a=12),
                in1=_ap(w_t[:], [[1, 12], [0, 8]]), op=AL.mult)
            nc.vector.tensor_tensor(out=dp[:], in0=dp[:], in1=dp[:], op=AL.mult)
            nc.vector.tensor_tensor(out=dp[:], in0=dp[:], in1=xy2[:], op=AL.add)
            # bit-mask d2 with bone visibility so sqrt never sees garbage
            nc.vector.tensor_tensor(out=dp[:].bitcast(_I32),
                                    in0=dp[:].bitcast(_I32), in1=vmi[:],
                                    op=AL.bitwise_and)
            # gt = (d2 > 0) on vector, overlapping the sqrt on scalar
            gt = pool.tile([P, 96], _F32)
            nc.vector.tensor_scalar(out=gt[:], in0=dp[:], scalar1=0.0,
                                    scalar2=None, op0=AL.is_gt)
            ell = pool.tile([P, 96], _F32)
            nc.scalar.sqrt(out=ell[:], in_=dp[:])
            # per-group mean E = sum_l / max(num,1) via reciprocal
            sum_l = pool.tile([P, 32], _F32)
            nc.vector.tensor_reduce(
                out=_ap(sum_l[:], [[8, 2], [1, 8]]),
                in_=_ap(ell[:], [[32, 2], [1, 8], [8, 4]]),
                axis=X, op=AL.add)
            nc.vector.tensor_reduce(
                out=_ap(sum_l[:], [[8, 2], [1, 8]], off=16),
                in_=_ap(ell[:], [[16, 2], [1, 8], [8, 2]], off=64),
                axis=X, op=AL.add)
            e_t = pool.tile([P, 32], _F32)
            nc.vector.tensor_tensor(out=e_t[:], in0=sum_l[:], in1=rn[:],
                                    op=AL.mult)
            # contrib = (ell>0) * (ell-E)^2 / num; global *0.5 on host.
            # E broadcast folded into the subtract via 0-stride APs.
            eb = pool.tile([P, 96], _F32)
            nc.vector.tensor_tensor(
                out=_ap(eb[:], [[32, 2], [8, 4], [1, 8]]),
                in0=_ap(ell[:], [[32, 2], [8, 4], [1, 8]]),
                in1=_ap(e_t[:], [[8, 2], [0, 4], [1, 8]]), op=AL.subtract)
            nc.vector.tensor_tensor(
                out=_ap(eb[:], [[16, 2], [8, 2], [1, 8]], off=64),
                in0=_ap(ell[:], [[16, 2], [8, 2], [1, 8]], off=64),
                in1=_ap(e_t[:], [[8, 2], [0, 2], [1, 8]], off=16),
                op=AL.subtract)
            nc.vector.tensor_tensor(out=eb[:], in0=eb[:], in1=eb[:], op=AL.mult)
            nc.vector.tensor_tensor(out=eb[:], in0=eb[:], in1=nb_b[:],
                                    op=AL.mult)
            # final mult by gt with free per-partition row sum
            pl1 = pool.tile([P, 1], _F32)
            nc.vector.scalar_tensor_tensor(
                out=eb[:], in0=eb[:], scalar=1.0, in1=gt[:],
                op0=AL.mult, op1=AL.mult, accum_out=pl1[:])
            ps = psum_pool.tile([1, 1], _F32, space="PSUM")
            nc.tensor.matmul(out=ps[:], lhsT=one_t[:], rhs=pl1[:],
                             start=True, stop=True)
            tot = pool.tile([1, 1], _F32)
            nc.vector.tensor_copy(out=tot[:], in_=ps[:])
            nc.sync.dma_start(res[:], tot[0:1, :])
    nc.compile()
    return nc


_NC_CACHE = None
LAST_RESULTS = None


def kernel(output, mask, ind, target, gt_2d):
    global _NC_CACHE, LAST_RESULTS
    if _NC_CACHE is None:
        _NC_CACHE = _build_nc()
    nc = _NC_CACHE

    output = np.ascontiguousarray(np.asarray(output), dtype=np.float32)
    mask = np.ascontiguousarray(np.asarray(mask), dtype=np.float32)
    target = np.ascontiguousarray(np.asarray(target), dtype=np.float32)
    gt_2d = np.ascontiguousarray(np.asarray(gt_2d), dtype=np.float32)
    ind = np.ascontiguousarray(np.asarray(ind))
    if ind.dtype != np.int64:
        ind = ind.astype(np.int64)

    consts = _consts()
    in_maps = []
    for c in range(NCORES):
        sl = slice(c * S, (c + 1) * S)
        in_maps.append({
            "outv": np.ascontiguousarray(output[sl]).reshape(S * 4096, 1),
            "indv": np.ascontiguousarray(ind[sl]).view(np.int32).reshape(S, 34),
            "tgtv": np.ascontiguousarray(target[sl]),
            "gxyv": np.ascontiguousarray(gt_2d[sl]).reshape(S, 34),
            "mskv": np.ascontiguousarray(mask[sl]),
            **consts,
        })
    res = run_bass_kernel_spmd(nc, in_maps, core_ids=list(range(NCORES)))
    LAST_RESULTS = res
    total = sum(float(res.results[c]["res"][0, 0]) for c in range(NCORES))
    return np.asarray([_VAR_WEIGHT * total * 0.5 / B], dtype=np.float32)
